# revision 1
# baseline (speedup 1.0000x reference)
"""Multi-head attention Trainium2 Bass kernel, 8-way SPMD.

Problem: nn_MultiHeadAttention (B=2, S=4096, D=512, H=8, Dk=64), fp32 I/O.

Sharding: the 8192 (B*S) query rows are split into 8 shards of 1024 rows,
one per NeuronCore (core c takes batch c//4, rows (c%4)*1024..). Each core
holds the full key/value sequence of its batch, so there are no
collectives; the per-core output rows concatenate into the full output.

Per-core algorithm (all matmuls contract over the partition axis):
  - PE-transpose raw inputs tile-wise to get feature-on-partition layouts.
  - Q^T = Wq @ xq^T + bq, K^T = Wk @ xk^T + bk   (fp32r matmuls)
  - V  = xv @ Wv^T + bv stored bf16 with a ones-column appended per head
    (V' = [V_h | 1]), so the AV matmul also produces the softmax
    denominator in PSUM row 64.
  - scores^T[k,q] = lhsT(K^T).T @ Q^T in PSUM; exp via ScalarE with
    scale=1/8 folded in (no max-subtraction: scores ~ N(0,1), exp is safe
    in fp32), output bf16 to SBUF. Head pairs share PE row-groups
    (tile_position auto-derived from base partitions 0/64).
  - attended^T + sums = V'.T @ exp(scores^T)  accumulated over k-chunks.
  - normalize with 1/sum broadcast along partitions via a PE ones-matmul,
  - out = attended^T.T @ Wo^T + bo, DMA'd to DRAM row-natural.

K^T / V' production is interleaved chunk-by-chunk into the first
attention head-pair loop so the exp stream on ScalarE starts immediately
and hides the projection work.

repeat>1 builds unroll the whole kernel for delta-timing; timing=True
builds replace the x inputs with device-initialized Internal DRAM so the
per-call staging overhead (~178MB over the axon tunnel) disappears from
measurements.
"""

from contextlib import ExitStack

import numpy as np

B = 2
S = 4096
D = 512
H = 8
DK = 64
P = 128
N_CORES = 8
SQ = (B * S) // N_CORES  # 1024 query rows per core
SKV = S  # 4096 kv rows per core
FC = D // P  # 4 feature chunks
NKT = SKV // P  # 32 key tiles
NSC = SKV // 512  # 8 seq chunks
QH = SQ // 512  # 2 query halves
INV_SCALE = 0.125  # 1/sqrt(DK)

_CACHE = {}


def _build_nc(repeat: int = 1, timing: bool = False, loop: int = 1):
    import concourse.mybir as mybir
    import concourse.tile as tile
    from concourse import bacc
    from concourse.masks import make_identity

    f32 = mybir.dt.float32
    f32r = mybir.dt.float32r
    bf16 = mybir.dt.bfloat16
    EXP = mybir.ActivationFunctionType.Exp

    nc = bacc.Bacc(
        "TRN2",
        target_bir_lowering=False,
        debug=False,
        enable_asserts=False,
        num_devices=N_CORES,
    )

    def din(name, shape):
        kind = "Internal" if timing and name in ("xq", "xk", "xv") else "ExternalInput"
        return nc.dram_tensor(name, shape, f32, kind=kind).ap()

    xq = din("xq", [SQ, D])
    xk = din("xk", [SKV, D])
    xv = din("xv", [SKV, D])
    wq, bq = din("wq", [D, D]), din("bq", [1, D])
    wk, bk = din("wk", [D, D]), din("bk", [1, D])
    wv, bv = din("wv", [D, D]), din("bv", [1, D])
    wo, bo = din("wo", [D, D]), din("bo", [1, D])
    out = nc.dram_tensor("out", [SQ, D], f32, kind="ExternalOutput").ap()

    from contextlib import nullcontext

    with tile.TileContext(nc) as tc:
      if timing:
        # deterministically fill the Internal x tensors (once per call,
        # outside the timing loop)
        with tc.tile_pool(name="init", bufs=1) as initp:
            fill = initp.tile([P, D], f32, name="fill")
            nc.vector.memset(fill, 0.01)
            for t_ap, rows in ((xq, SQ), (xk, SKV), (xv, SKV)):
                for rt in range(rows // P):
                    nc.sync.dma_start(t_ap[rt * P : (rt + 1) * P, :], fill)

      with tc.For_i(0, loop, 1) if loop > 1 else nullcontext():
       for rep in range(repeat):
         sx = f"_r{rep}" if repeat > 1 else ""
         st = ExitStack()
         consts = st.enter_context(tc.tile_pool(name=f"consts{sx}", bufs=1))
         ident = consts.tile([P, P], f32, name=f"ident{sx}")
         make_identity(nc, ident)
         ones = consts.tile([1, D], f32r, name=f"ones{sx}")
         bias_t = {}
         with tc.tile_pool(name=f"bstage{sx}", bufs=2) as bstage:
             ones_stg = bstage.tile([1, D], f32, tag="bstg", name=f"ones_stg{sx}")
             nc.vector.memset(ones_stg, 1.0)
             nc.vector.tensor_copy(ones, ones_stg)
             for name, ap in [("bq", bq), ("bk", bk), ("bv", bv), ("bo", bo)]:
                 stg = bstage.tile([1, D], f32, tag="bstg", name=f"stg_{name}{sx}")
                 nc.sync.dma_start(stg, ap)
                 t = consts.tile([1, D], f32r, tag=f"bias_{name}", name=f"b_{name}{sx}")
                 nc.vector.tensor_copy(t, stg)
                 bias_t[name] = t

         # bias columns [128, 4]: partition = d within tile, col = d-tile
         bcol = {}
         for name, ap in [("bq", bq), ("bk", bk)]:
             t = consts.tile([P, FC], f32, tag=f"bcol_{name}", name=f"bc_{name}{sx}")
             nc.sync.dma_start(t, ap.rearrange("o (t p) -> p (o t)", p=P))
             bcol[name] = t
         bvb = consts.tile([P, D], f32, name=f"bvb{sx}")

         # ---- phase 1: transpose weights (w^T[f, j] = w[j, f]) ----
         wT_pool = st.enter_context(tc.tile_pool(name=f"wT{sx}", bufs=1))
         wT = {}

         def emit_wT(wname, w_ap, pool, wload, tpsum):
             wt = pool.tile([P, FC, D], f32r, tag=f"{wname}T", name=f"{wname}T{sx}")
             wT[wname] = wt
             for rt in range(FC):
                 w_tile = wload.tile(
                     [P, D], f32, tag="wld", name=f"wld_{wname}{rt}{sx}"
                 )
                 nc.sync.dma_start(w_tile, w_ap[rt * P : (rt + 1) * P, :])
                 pst = tpsum.tile(
                     [P, FC, P], f32, tag="wpst", name=f"wpst{wname}{rt}{sx}"
                 )
                 for fc in range(FC):
                     nc.tensor.transpose(
                         pst[:, fc, :], w_tile[:, fc * P : (fc + 1) * P], ident
                     )
                 nc.vector.tensor_copy(wt[:, :, rt * P : (rt + 1) * P], pst)

         with (
             tc.tile_pool(name=f"wload{sx}", bufs=2) as wload,
             tc.tile_pool(name=f"tpsum1{sx}", bufs=4, space="PSUM") as tpsum,
         ):
             for wname, w_ap in [("wk", wk), ("wv", wv), ("wo", wo)]:
                 emit_wT(wname, w_ap, wT_pool, wload, tpsum)
             ps_bvb = tpsum.tile([P, D], f32, tag="bvb", name=f"psbvb{sx}")
             nc.tensor.matmul(
                 ps_bvb, lhsT=ones[:, 0:P], rhs=bias_t["bv"], start=True, stop=True
             )
             nc.vector.tensor_copy(bvb, ps_bvb)

         # ---- phase 2: Q^T [D, SQ] ----
         qT_pool = st.enter_context(tc.tile_pool(name=f"QT{sx}", bufs=1))
         QT = [
             qT_pool.tile([P, SQ], f32r, tag=f"QT{dt}", name=f"QT{dt}{sx}")
             for dt in range(FC)
         ]
         with (
             tc.tile_pool(name=f"xload2{sx}", bufs=3) as xload,
             tc.tile_pool(name=f"xqT{sx}", bufs=1) as xT_pool,
             tc.tile_pool(name=f"tpsum2{sx}", bufs=2, space="PSUM") as tpsum,
             tc.tile_pool(name=f"ppsum2{sx}", bufs=2, space="PSUM") as ppsum,
         ):
             emit_wT("wq", wq, xT_pool, xload, tpsum)
             xqT = xT_pool.tile([P, FC, SQ], f32r, name=f"xqT{sx}")
             for rt in range(SQ // P):
                 x_tile = xload.tile([P, D], f32, tag="xql", name=f"xql{rt}{sx}")
                 nc.sync.dma_start(x_tile, xq[rt * P : (rt + 1) * P, :])
                 pst = tpsum.tile([P, FC, P], f32, tag="qpst", name=f"qpst{rt}{sx}")
                 for fc in range(FC):
                     nc.tensor.transpose(
                         pst[:, fc, :], x_tile[:, fc * P : (fc + 1) * P], ident
                     )
                 nc.vector.tensor_copy(xqT[:, :, rt * P : (rt + 1) * P], pst)
             for dt in range(FC):
                 for qh in range(QH):
                     ps = ppsum.tile([P, 512], f32, tag="qps", name=f"qps{dt}{qh}{sx}")
                     for fc in range(FC):
                         nc.tensor.matmul(
                             ps,
                             lhsT=wT["wq"][:, fc, dt * P : (dt + 1) * P],
                             rhs=xqT[:, fc, qh * 512 : (qh + 1) * 512],
                             start=(fc == 0),
                             stop=(fc == FC - 1),
                         )
                     nc.vector.tensor_scalar_add(
                         QT[dt][:, qh * 512 : (qh + 1) * 512],
                         in0=ps,
                         scalar1=bcol["bq"][:, dt : dt + 1],
                     )

         # ---- resident K^T [D, SKV] and V' [128, NKT, H, 65] bf16 ----
         kT_pool = st.enter_context(tc.tile_pool(name=f"KT{sx}", bufs=1))
         KT = [
             kT_pool.tile([P, SKV], f32r, tag=f"KT{dt}", name=f"KT{dt}{sx}")
             for dt in range(FC)
         ]
         vp_pool = st.enter_context(tc.tile_pool(name=f"Vp{sx}", bufs=1))
         Vp = vp_pool.tile([P, NKT, H, DK + 1], bf16, name=f"Vp{sx}")

         # ---- attention pools (persistent PSUM: spsum 4 + attacc 2 banks) ----
         att_st = ExitStack()
         exp_pool = att_st.enter_context(tc.tile_pool(name=f"exp{sx}", bufs=3))
         spsum = att_st.enter_context(
             tc.tile_pool(name=f"spsum{sx}", bufs=2, space="PSUM")
         )
         attacc = att_st.enter_context(
             tc.tile_pool(name=f"attacc{sx}", bufs=1, space="PSUM")
         )

         # production pools for K^T/V' chunks (tpsum 1 + ppsum 1 bank),
         # open only during the first head-pair loop
         prod_st = ExitStack()
         pxload = prod_st.enter_context(tc.tile_pool(name=f"pxload{sx}", bufs=2))
         xTs_pool = prod_st.enter_context(tc.tile_pool(name=f"xTs{sx}", bufs=2))
         ptpsum = prod_st.enter_context(
             tc.tile_pool(name=f"ptpsum{sx}", bufs=1, space="PSUM")
         )
         pppsum = prod_st.enter_context(
             tc.tile_pool(name=f"pppsum{sx}", bufs=1, space="PSUM")
         )

         def chunk_steps(sc):
             """Production steps for K^T[:, sc*512:..] and Vp[:, 4sc:4sc+4],
             as thunks so they can be interleaved with attention work."""
             steps = []
             for part, x_ap in enumerate([xk, xv]):
                 xTs = xTs_pool.tile(
                     [P, FC, 512], f32r, tag="xTs", name=f"xTs{part}_{sc}{sx}"
                 )
                 x_big = pxload.tile(
                     [P, 4, D], f32, tag="xbig", name=f"xb{part}_{sc}{sx}"
                 )

                 def load(x_ap=x_ap, x_big=x_big):
                     nc.sync.dma_start(
                         x_big,
                         x_ap[sc * 512 : (sc + 1) * 512, :].rearrange(
                             "(rt p) d -> p rt d", p=P
                         ),
                     )

                 steps.append(load)

                 def trans(rt, part=part, x_big=x_big, xTs=xTs):
                     pst = ptpsum.tile(
                         [P, FC, P], f32, tag="pst", name=f"pst{part}_{sc}_{rt}{sx}"
                     )
                     for fc in range(FC):
                         nc.tensor.transpose(
                             pst[:, fc, :],
                             x_big[:, rt, fc * P : (fc + 1) * P],
                             ident,
                         )
                     nc.vector.tensor_copy(xTs[:, :, rt * P : (rt + 1) * P], pst)

                 for rt in range(4):
                     steps.append(lambda rt=rt, f=trans: f(rt))
                 if part == 0:

                     def kproj(dt, xTs=xTs):
                         ps = pppsum.tile(
                             [P, 512], f32, tag="pps", name=f"kps{sc}_{dt}{sx}"
                         )
                         for fc in range(FC):
                             nc.tensor.matmul(
                                 ps,
                                 lhsT=wT["wk"][:, fc, dt * P : (dt + 1) * P],
                                 rhs=xTs[:, fc, :],
                                 start=(fc == 0),
                                 stop=(fc == FC - 1),
                             )
                         nc.vector.tensor_scalar_add(
                             KT[dt][:, sc * 512 : (sc + 1) * 512],
                             in0=ps,
                             scalar1=bcol["bk"][:, dt : dt + 1],
                         )

                     for dt in range(FC):
                         steps.append(lambda dt=dt, f=kproj: f(dt))
                 else:

                     def vproj(vt, xTs=xTs):
                         kt = sc * 4 + vt
                         ps = pppsum.tile(
                             [P, 512], f32, tag="pps", name=f"vps{sc}_{vt}{sx}"
                         )
                         for fc in range(FC):
                             nc.tensor.matmul(
                                 ps,
                                 lhsT=xTs[:, fc, vt * P : (vt + 1) * P],
                                 rhs=wT["wv"][:, fc, :],
                                 start=(fc == 0),
                                 stop=(fc == FC - 1),
                             )
                         nc.vector.tensor_add(
                             Vp[:, kt, :, 0:DK],
                             ps.rearrange("p (h d) -> p h d", h=H),
                             bvb.rearrange("p (h d) -> p h d", h=H),
                         )
                         if vt == 3:
                             nc.vector.memset(
                                 Vp[:, sc * 4 : (sc + 1) * 4, :, DK : DK + 1], 1.0
                             )

                     for vt in range(4):
                         steps.append(lambda vt=vt, f=vproj: f(vt))
             return steps

         # ---- attention + output projection ----
         first_loop = True
         opsum = attT_pool = small = outbuf = None
         op_st = ExitStack()
         for qh in range(QH):
             qs = slice(qh * 512, (qh + 1) * 512)
             attT_t = None
             for p in range(H // 2):  # head pair (2p, 2p+1)
                 acc = [
                     attacc.tile(
                         [DK + 1, 512], f32, tag=f"acc{i}", name=f"acc{qh}_{p}_{i}{sx}"
                     )
                     for i in range(2)
                 ]
                 if first_loop:
                     # pre-seed two chunks, then drain the queue at a rate
                     # that keeps production >= 1 chunk ahead of consumption
                     queue = []
                     for sc in range(NSC):
                         queue.extend(chunk_steps(sc))
                     pos = 0
                     for _ in range(36):  # chunks 0 and 1 fully
                         queue[pos]()
                         pos += 1
                 for kt in range(NKT):
                     if first_loop:
                         # keep production 2 chunks ahead: chunk kt//4+2
                         # completes within the current 4-kt window
                         target = min(len(queue), 18 * (kt // 4 + 3))
                         while pos < target:
                             queue[pos]()
                             pos += 1
                     ks = slice(kt * P, (kt + 1) * P)
                     sc_ps = spsum.tile(
                         [P, 2, 512], f32, tag="sc", name=f"sc{qh}_{p}_{kt}{sx}"
                     )
                     nc.tensor.matmul(
                         sc_ps[:, 0, :],
                         lhsT=KT[p][0:DK, ks],
                         rhs=QT[p][0:DK, qs],
                         start=True,
                         stop=True,
                     )
                     nc.tensor.matmul(
                         sc_ps[:, 1, :],
                         lhsT=KT[p][DK:P, ks],
                         rhs=QT[p][DK:P, qs],
                         start=True,
                         stop=True,
                     )
                     ex = exp_pool.tile(
                         [P, 2, 512], bf16, tag="ex", name=f"ex{qh}_{p}_{kt}{sx}"
                     )
                     nc.scalar.activation(ex, sc_ps, func=EXP, scale=INV_SCALE)
                     for i in range(2):
                         nc.tensor.matmul(
                             acc[i],
                             lhsT=Vp[:, kt, 2 * p + i, :],
                             rhs=ex[:, i, :],
                             start=(kt == 0),
                             stop=(kt == NKT - 1),
                         )
                 if first_loop:
                     # production done; swap production pools for out-proj pools
                     prod_st.close()
                     opsum = op_st.enter_context(
                         tc.tile_pool(name=f"opsum{sx}", bufs=2, space="PSUM")
                     )
                     attT_pool = op_st.enter_context(
                         tc.tile_pool(name=f"attT{sx}", bufs=2)
                     )
                     small = op_st.enter_context(
                         tc.tile_pool(name=f"small{sx}", bufs=2)
                     )
                     outbuf = op_st.enter_context(
                         tc.tile_pool(name=f"outbuf{sx}", bufs=2)
                     )
                     first_loop = False
                 if attT_t is None:
                     attT_t = attT_pool.tile(
                         [P, FC, 512], f32r, tag="attT", name=f"attT{qh}{sx}"
                     )
                 for i in range(2):
                     h = 2 * p + i
                     acc_sb = small.tile(
                         [DK + 1, 512], f32, tag="acc_sb", name=f"asb{qh}_{h}{sx}"
                     )
                     nc.vector.tensor_copy(acc_sb, acc[i])
                     rc = small.tile([1, 512], f32r, tag="rc", name=f"rc{qh}_{h}{sx}")
                     with nc.allow_low_precision(reason="f32r recip softmax denom"):
                         nc.vector.reciprocal(rc, acc_sb[DK : DK + 1, :])
                     ps_rb = opsum.tile(
                         [DK, 512], f32, tag="po", name=f"psrb{qh}_{h}{sx}"
                     )
                     nc.tensor.matmul(
                         ps_rb, lhsT=ones[:, 0:DK], rhs=rc, start=True, stop=True
                     )
                     rb = small.tile([DK, 512], f32, tag="rb", name=f"rb{qh}_{h}{sx}")
                     nc.vector.tensor_copy(rb, ps_rb)
                     nc.vector.tensor_mul(
                         attT_t[(h % 2) * DK : (h % 2 + 1) * DK, h // 2, :],
                         acc_sb[0:DK, :],
                         rb,
                     )
             # output projection for this q half
             for qt in range(4):
                 po = opsum.tile([P, D], f32, tag="po", name=f"po{qh}_{qt}{sx}")
                 for dt in range(FC):
                     nc.tensor.matmul(
                         po,
                         lhsT=attT_t[:, dt, qt * P : (qt + 1) * P],
                         rhs=wT["wo"][:, dt, :],
                         start=(dt == 0),
                         stop=False,
                     )
                 nc.tensor.matmul(
                     po,
                     lhsT=ones[:, 0:P],
                     rhs=bias_t["bo"],
                     start=False,
                     stop=True,
                 )
                 ot = outbuf.tile([P, D], f32, tag="ot", name=f"ot{qh}_{qt}{sx}")
                 nc.vector.tensor_copy(ot, po)
                 nc.sync.dma_start(
                     out[qh * 512 + qt * P : qh * 512 + (qt + 1) * P, :], ot
                 )
         op_st.close()
         att_st.close()
         st.close()

    nc.compile()
    return nc


def get_nc(repeat: int = 1, timing: bool = False, loop: int = 1):
    key = f"nc{repeat}{'t' if timing else ''}l{loop}"
    if key not in _CACHE:
        _CACHE[key] = _build_nc(repeat, timing, loop)
    return _CACHE[key]


def make_in_maps(query, key, value, w_q, b_q, w_k, b_k, w_v, b_v, w_o, b_o):
    query = np.ascontiguousarray(np.asarray(query, dtype=np.float32)).reshape(
        B * S, D
    )
    key = np.asarray(key, dtype=np.float32)
    value = np.asarray(value, dtype=np.float32)
    shared = {
        "wq": np.ascontiguousarray(w_q, dtype=np.float32),
        "bq": np.ascontiguousarray(b_q, dtype=np.float32).reshape(1, D),
        "wk": np.ascontiguousarray(w_k, dtype=np.float32),
        "bk": np.ascontiguousarray(b_k, dtype=np.float32).reshape(1, D),
        "wv": np.ascontiguousarray(w_v, dtype=np.float32),
        "bv": np.ascontiguousarray(b_v, dtype=np.float32).reshape(1, D),
        "wo": np.ascontiguousarray(w_o, dtype=np.float32),
        "bo": np.ascontiguousarray(b_o, dtype=np.float32).reshape(1, D),
    }
    in_maps = []
    for c in range(N_CORES):
        b = c // (N_CORES // B)
        r0 = (c % (N_CORES // B)) * SQ
        in_maps.append(
            {
                "xq": query[b * S + r0 : b * S + r0 + SQ, :],
                "xk": np.ascontiguousarray(key[b]),
                "xv": np.ascontiguousarray(value[b]),
                **shared,
            }
        )
    return in_maps


def kernel(query, key, value, w_q, b_q, w_k, b_k, w_v, b_v, w_o, b_o):
    from concourse import bass_utils

    in_maps = make_in_maps(
        query, key, value, w_q, b_q, w_k, b_k, w_v, b_v, w_o, b_o
    )
    nc = get_nc()
    res = bass_utils.run_bass_kernel_spmd(nc, in_maps, core_ids=list(range(N_CORES)))
    out = np.concatenate([res.results[c]["out"] for c in range(N_CORES)], axis=0)
    return out.reshape(B, S, D)


if __name__ == "__main__":
    nc = get_nc()
    print("built ok")

